# revision 22
# baseline (speedup 1.0000x reference)
"""GCNConv-pair (mu/logstd) message-passing kernel for 8 trn2 NeuronCores.

Strategy:
  - Host: fold sym-norm dinv into x rows; partition edges by destination
    core; bucket into 128-dst windows; split lo/hi by (padded) source row
    so gather indices fit int16; pad per (window, view) to the max count
    across cores so the SPMD program is uniform.
  - Device (SPMD, identical program, per-core data):
      Phase 1: hs_slice = (dinv*x_slice) @ [W_mu | W_logstd] (bf16 matmul
               over this core's 6250 nodes), AllGather -> full hs table.
      Phase 2: per (window, view) dma_gather of hs rows (4 SWDGE queues in
               parallel), one-hot chunk matmuls (TensorE) accumulate the
               segment sum per 128-dst window in PSUM, fused epilogue
               out = psum * dinv_dst + bias, DMA to outputs.
"""

import os
import sys

sys.path.insert(0, "/opt/trn_rl_repo")

import numpy as np
import ml_dtypes

import concourse.bass as bass
import concourse.bacc as bacc
import concourse.tile as tile
from concourse import mybir
from concourse.bass_utils import run_bass_kernel_spmd

# ---- problem constants (hardcoded per harness contract) ----
N_NODES = 50000
N_EDGES = 800000
IN_SIZE = 512
OUT_SIZE = 128
DOUT = 2 * OUT_SIZE  # 256, mu|logstd concatenated
NCORES = 8
NPC = N_NODES // NCORES  # 6250 nodes per core
NW = (NPC + 127) // 128  # 49 windows of 128 dst nodes
LASTW_ROWS = NPC - (NW - 1) * 128  # 106
NPC_PAD = NW * 128  # 6272 padded rows per core in the hs table
NPAD = NPC_PAD * NCORES  # 50176
SPLIT = 25088  # lo/hi padded-row split for int16 gather indices
NQ = 4  # SWDGE queues used round-robin for gathers
SAFE_W = 8  # first windows per view gather full padded tiles (init SBUF)

BF16 = ml_dtypes.bfloat16
LAST_RESULT = None


def _install_ntff_shim():
    """Register the axon NTFF profile hook if the glue module is missing."""
    try:
        import contextlib
        import ctypes
        import types

        import antenv  # noqa: F401

        if "antenv.axon_hooks" in sys.modules:
            return
        so_path = "/opt/axon/libaxon_pjrt.so"
        try:
            lib = ctypes.CDLL(so_path)
        except OSError:
            return
        if not hasattr(lib, "axon_start_nrt_profile"):
            return
        lib.axon_start_nrt_profile.argtypes = [
            ctypes.POINTER(ctypes.c_int64),
            ctypes.c_size_t,
        ]
        lib.axon_start_nrt_profile.restype = ctypes.c_int64
        lib.axon_stop_nrt_profile.argtypes = [ctypes.c_char_p]
        lib.axon_stop_nrt_profile.restype = ctypes.c_int64

        @contextlib.contextmanager
        def _hook(output_dir, device_ids):
            import jax

            jax.devices()
            if device_ids:
                ids = (ctypes.c_int64 * len(device_ids))(*device_ids)
                rc = lib.axon_start_nrt_profile(ids, len(device_ids))
            else:
                rc = lib.axon_start_nrt_profile(None, 0)
            if rc != 0:
                raise RuntimeError(f"axon_start_nrt_profile rc={rc}")
            try:
                yield
            finally:
                n = lib.axon_stop_nrt_profile(str(output_dir).encode())
                if n < 0:
                    raise RuntimeError(f"axon_stop_nrt_profile rc={n}")

        hook = _hook
        mod = types.ModuleType("antenv.axon_hooks")
        mod.set_axon_ntff_profile_hook = lambda h: None
        mod.get_axon_ntff_profile_hook = lambda: hook
        sys.modules["antenv.axon_hooks"] = mod
        antenv.axon_hooks = mod
    except Exception:
        pass


def _prep_edges(edge_index):
    """Host-side edge partitioning. Returns per-core gather structures.

    Self-loops (PyG default) are NOT part of the gathered message stream;
    they are folded into the epilogue on-device (psum += dinv*h of the own
    window, read back from the local hs bounce buffer).
    """
    src = np.asarray(edge_index[0], dtype=np.int64)
    dst = np.asarray(edge_index[1], dtype=np.int64)

    # degree includes the self-loop (PyG GCNConv adds them)
    deg = (np.bincount(dst, minlength=N_NODES) + 1).astype(np.float64)
    dinv = (1.0 / np.sqrt(np.maximum(deg, 1.0))).astype(np.float32)

    core = dst // NPC
    dstl = dst - core * NPC
    win = dstl >> 7
    slot = (dstl & 127).astype(np.float32)
    # padded hs-table row of the source node
    srow = src + 22 * (src // NPC)
    view = (srow >= SPLIT).astype(np.int64)
    idx16 = np.where(view == 0, srow, srow - SPLIT)

    flat_key = (core * 2 + view) * NW + win
    cnt = np.bincount(flat_key, minlength=NCORES * 2 * NW)
    counts = cnt.reshape(NCORES, 2, NW)

    # per (view, window) gather length: max count across cores
    rwv = counts.max(axis=0)  # [2, NW]
    kwv = (rwv + 127) // 128  # chunks per (view, window)
    # chunk offsets per view (for slot arrays / matmul indexing)
    chunk_off = np.zeros((2, NW + 1), np.int64)
    chunk_off[:, 1:] = np.cumsum(kwv, axis=1)
    n_chunks = chunk_off[:, -1]  # per view
    # idx column offsets per view: full chunks (128 idx = 8 cols of 16)
    col_off = chunk_off * 8
    n_cols = col_off[:, -1]

    idx_arrs = [
        np.zeros((NCORES, 128, int(n_cols[v])), np.int16) for v in range(2)
    ]
    slot_arrs = [
        np.full((NCORES, 128, int(n_chunks[v])), -1.0, np.float32)
        for v in range(2)
    ]

    order = np.lexsort((win, view, core))
    s_src = idx16[order]
    s_slot = slot[order]
    s_key = flat_key[order]
    bucket_start = np.zeros(NCORES * 2 * NW + 1, np.int64)
    bucket_start[1:] = np.cumsum(cnt)
    rank = np.arange(src.size) - bucket_start[s_key]
    s_core = s_key // (2 * NW)
    s_view = (s_key // NW) % 2
    s_win = s_key % NW

    # per-core valid message counts per (view, window); pads beyond the
    # valid prefix are -1 (the gather ucode trims trailing negatives and
    # generates no descriptors for them). The first SAFE_W windows instead
    # keep 0-padding and gather the full padded tile so the rotating msg
    # buffers start fully initialized (stale SBUF could hold NaN patterns).
    nvalid = np.zeros((NCORES, 2 * NW), np.int32)
    for v in range(2):
        S_idx = int(n_cols[v]) * 16
        S_slot = int(n_chunks[v]) * 128
        idxpos = chunk_off[v][s_win] * 128 + rank
        slotpos = idxpos
        # default pad value by window of each position
        pos_win = (
            np.searchsorted(chunk_off[v] * 128, np.arange(S_idx), side="right") - 1
        )
        default_idx = np.where(pos_win < SAFE_W, 0, -1).astype(np.int64)
        for c in range(NCORES):
            m = (s_core == c) & (s_view == v)
            idx_flat = default_idx.copy()
            idx_flat[idxpos[m]] = s_src[m]
            # wrapped: idx i of a window lives at [i%16, coloff + i//16];
            # since window offsets are 16-aligned this is a global reshape
            w16 = idx_flat.reshape(-1, 16).T.astype(np.int16)  # [16, S/16]
            idx_arrs[v][c] = np.tile(w16, (8, 1))
            slot_flat = np.full(S_slot, -1.0, np.float32)
            slot_flat[slotpos[m]] = s_slot[m]
            slot_arrs[v][c] = slot_flat.reshape(-1, 128).T
            cnts = counts[c, v]  # [NW]
            nvalid[c, v * NW : (v + 1) * NW] = np.where(
                np.arange(NW) < SAFE_W, kwv[v] * 128, cnts
            )

    dinv_out = np.zeros((NCORES, 128, NW), np.float32)
    for c in range(NCORES):
        d = np.zeros(NW * 128, np.float32)
        d[:NPC] = dinv[c * NPC : (c + 1) * NPC]
        dinv_out[c] = d.reshape(NW, 128).T

    return {
        "dinv": dinv,
        "rwv": rwv,
        "kwv": kwv,
        "chunk_off": chunk_off,
        "col_off": col_off,
        "n_chunks": n_chunks,
        "n_cols": n_cols,
        "idx_arrs": [a.astype(np.int16) for a in idx_arrs],
        "slot_arrs": [a.astype(BF16) for a in slot_arrs],
        "dinv_out": dinv_out,
        "nvalid": nvalid,
    }


def _build_program(prep):
    """Build the SPMD bass program (identical across cores)."""
    rwv = prep["rwv"]
    kwv = prep["kwv"]
    chunk_off = prep["chunk_off"]
    col_off = prep["col_off"]
    n_chunks = prep["n_chunks"]
    n_cols = prep["n_cols"]

    nc = bacc.Bacc(
        "TRN2",
        target_bir_lowering=False,
        debug=False,
        num_devices=NCORES,
        num_swdge_queues=NQ,
    )
    bf16 = mybir.dt.bfloat16
    f32 = mybir.dt.float32
    i16 = mybir.dt.int16
    i32 = mybir.dt.int32

    t_xsT = nc.dram_tensor("xsT", [4, 128, NPC_PAD], bf16, kind="ExternalInput")
    t_w = nc.dram_tensor("wcat", [4, 128, DOUT], bf16, kind="ExternalInput")
    t_bias = nc.dram_tensor("bias", [128, DOUT], f32, kind="ExternalInput")
    t_iota = nc.dram_tensor("iota", [128, 128], bf16, kind="ExternalInput")
    t_idx0 = nc.dram_tensor("idx0", [128, int(n_cols[0])], i16, kind="ExternalInput")
    t_idx1 = nc.dram_tensor("idx1", [128, int(n_cols[1])], i16, kind="ExternalInput")
    t_slot0 = nc.dram_tensor("slot0", [128, int(n_chunks[0])], bf16, kind="ExternalInput")
    t_slot1 = nc.dram_tensor("slot1", [128, int(n_chunks[1])], bf16, kind="ExternalInput")
    t_dinv = nc.dram_tensor("dinv_out", [128, NW], f32, kind="ExternalInput")
    t_nvalid = nc.dram_tensor("nvalid", [128, 2 * NW], i32, kind="ExternalInput")
    t_mu = nc.dram_tensor("out_mu", [NPC, OUT_SIZE], f32, kind="ExternalOutput")
    t_ls = nc.dram_tensor("out_ls", [NPC, OUT_SIZE], f32, kind="ExternalOutput")
    t_idx = [t_idx0, t_idx1]
    t_slot = [t_slot0, t_slot1]

    with tile.TileContext(nc) as tc:
        with (
            tc.tile_pool(name="dram", bufs=1, space="DRAM") as dram,
            tc.tile_pool(name="const", bufs=1) as const,
            tc.tile_pool(name="hst", bufs=4) as hstp,
            tc.tile_pool(name="msg", bufs=8) as msgp,
            tc.tile_pool(name="hso", bufs=4) as hsop,
            tc.tile_pool(name="oh", bufs=4) as ohp,
            tc.tile_pool(name="res", bufs=4) as resp,
            tc.tile_pool(name="psum", bufs=4, space="PSUM") as psp,
        ):
            hs_bounce = dram.tile([NPC_PAD, DOUT], bf16)
            hs_full = dram.tile([NPAD, DOUT], bf16, name="hs_full", addr_space="Shared")

            # ---- Phase 1: hs_slice = xs @ Wcat, then AllGather ----
            with tc.tile_pool(name="xkp", bufs=1) as xkp:
                # phase-1-critical loads go first: x slices then weights
                xk = []
                for kt in range(4):
                    xt_ = xkp.tile([128, NPC_PAD], bf16, name=f"xk{kt}")
                    nc.sync.dma_start(xt_[:], t_xsT[kt])
                    xk.append(xt_)
                w_tiles = []
                for kt in range(4):
                    wt = const.tile([128, DOUT], bf16, name=f"w{kt}")
                    nc.sync.dma_start(wt[:], t_w[kt])
                    w_tiles.append(wt)
                # remaining constants (phase-2 only; loaded behind phase 1)
                bias_t = const.tile([128, DOUT], f32, name="bias_t")
                nc.sync.dma_start(bias_t[:], t_bias[:])
                iota_t = const.tile([128, 128], bf16, name="iota_t")
                nc.sync.dma_start(iota_t[:], t_iota[:])
                dinv_t = const.tile([128, NW], f32, name="dinv_t")
                nc.sync.dma_start(dinv_t[:], t_dinv[:])
                nv_t = const.tile([128, 2 * NW], i32, name="nv_t")
                nc.sync.dma_start(nv_t[:], t_nvalid[:])
                # full slot tables resident in SBUF (DVE reads strided slices)
                slot_c = []
                for v in range(2):
                    st = const.tile(
                        [128, int(n_chunks[v])], bf16, name=f"slotc{v}"
                    )
                    nc.sync.dma_start(st[:], t_slot[v][:])
                    slot_c.append(st)
                # full idx tables resident in SBUF (gather ucode reads slices)
                idx_c = []
                for v in range(2):
                    it = const.tile([128, int(n_cols[v])], i16, name=f"idxc{v}")
                    nc.sync.dma_start(it[:], t_idx[v][:])
                    idx_c.append(it)

                for g in range(0, NW, 4):
                    gn = min(4, NW - g)
                    hstage = hstp.tile([128, 4, DOUT], bf16, name="hstage")
                    for j in range(gn):
                        nb = g + j
                        ph = psp.tile([128, DOUT], f32, name="ph", tag="ph")
                        for kt in range(4):
                            nc.tensor.matmul(
                                out=ph[:],
                                lhsT=xk[kt][:, nb * 128 : (nb + 1) * 128],
                                rhs=w_tiles[kt][:],
                                start=(kt == 0),
                                stop=(kt == 3),
                            )
                        nc.vector.tensor_copy(hstage[:, j, :], ph[:])
                    nc.sync.dma_start(
                        hs_bounce[:]
                        .rearrange("(a p) d -> p a d", p=128)[:, g : g + gn, :],
                        hstage[:, :gn, :],
                    )
                # hs_bounce rows are node-major: row = a*128 + p
                nc.gpsimd.collective_compute(
                    "AllGather",
                    mybir.AluOpType.bypass,
                    replica_groups=[list(range(NCORES))],
                    ins=[hs_bounce[:].opt()],
                    outs=[hs_full[:].opt()],
                )

            hs_ap = hs_full[:]
            views = [hs_ap[0 : SPLIT + 128, :], hs_ap[SPLIT:NPAD, :]]

            # ---- Phase 2: per-(window,view) gathers + one-hot segment-sum ----
            # num_idxs_reg is loaded per-core from the nvalid table, so each
            # core only generates descriptors for its own valid messages
            # (trailing -1 indices are skipped by the gather ucode).
            nv_reg = nc.gpsimd.alloc_register("nv_reg")
            ohmax = [int(kwv[v].max()) for v in range(2)]
            qn = 0
            for w in range(NW):
                oh = [None, None]
                msgs = [None, None]
                for v in range(2):
                    ch0, ch1 = int(chunk_off[v][w]), int(chunk_off[v][w + 1])
                    kw = ch1 - ch0
                    oh[v] = ohp.tile(
                        [128, ohmax[v], 128], bf16, name=f"oh{v}", tag=f"oh{v}"
                    )
                    nc.vector.tensor_tensor(
                        out=oh[v][:, :kw, :],
                        in0=slot_c[v][:, ch0:ch1]
                        .unsqueeze(-1)
                        .broadcast_to([128, kw, 128]),
                        in1=iota_t[:].unsqueeze(1).broadcast_to([128, kw, 128]),
                        op=mybir.AluOpType.is_equal,
                    )
                    mt = msgp.tile(
                        [128, ohmax[v], DOUT], bf16, name=f"msg{v}", tag=f"msg{v}"
                    )
                    nc.gpsimd.reg_load(nv_reg, nv_t[0:1, v * NW + w : v * NW + w + 1])
                    nc.gpsimd.dma_gather(
                        out_ap=mt[:, :kw, :],
                        in_ap=views[v],
                        idxs_ap=idx_c[v][:, ch0 * 8 : ch1 * 8],
                        num_idxs=kw * 128,
                        num_idxs_reg=nv_reg,
                        elem_size=DOUT,
                        single_packet=False,
                        queue_num=qn,
                    )
                    qn = (qn + 1) % NQ
                    msgs[v] = mt
                # own-window hs rows for the self-loop term (local bounce)
                hso = hsop.tile([128, DOUT], bf16, name="hso", tag="hso")
                nc.sync.dma_start(hso[:], hs_bounce[w * 128 : (w + 1) * 128, :])
                po = psp.tile([128, DOUT], f32, name="po", tag="po")
                nmm = int(kwv[0][w] + kwv[1][w])
                i = 0
                for v in range(2):
                    for k in range(int(kwv[v][w])):
                        nc.tensor.matmul(
                            out=po[:],
                            lhsT=oh[v][:, k, :],
                            rhs=msgs[v][:, k, :],
                            start=(i == 0),
                            stop=(i == nmm - 1),
                        )
                        i += 1
                tmp = resp.tile([128, DOUT], f32, name="tmp", tag="tmp")
                nc.vector.tensor_tensor(
                    out=tmp[:], in0=po[:], in1=hso[:], op=mybir.AluOpType.add
                )
                res = resp.tile([128, DOUT], f32, name="res", tag="res")
                nc.vector.scalar_tensor_tensor(
                    out=res[:],
                    in0=tmp[:],
                    scalar=dinv_t[:, w : w + 1],
                    in1=bias_t[:],
                    op0=mybir.AluOpType.mult,
                    op1=mybir.AluOpType.add,
                )
                rows = LASTW_ROWS if w == NW - 1 else 128
                r0 = w * 128
                nc.sync.dma_start(t_mu[r0 : r0 + rows, :], res[:rows, 0:OUT_SIZE])
                nc.sync.dma_start(
                    t_ls[r0 : r0 + rows, :], res[:rows, OUT_SIZE:DOUT]
                )

    nc.compile()
    return nc


def kernel(x, edge_index, W_mu, b_mu, W_logstd, b_logstd):
    _install_ntff_shim()

    x = np.asarray(x, dtype=np.float32)
    prep = _prep_edges(np.asarray(edge_index))

    # fold dinv into x rows; per-core transposed slice [4,128,NPC_PAD]
    xs = (x * prep["dinv"][:, None]).astype(BF16)
    xsT_cores = []
    for c in range(NCORES):
        sl = np.zeros((IN_SIZE, NPC_PAD), BF16)
        sl[:, :NPC] = xs[c * NPC : (c + 1) * NPC].T
        xsT_cores.append(np.ascontiguousarray(sl.reshape(4, 128, NPC_PAD)))

    wcat = np.concatenate(
        [np.asarray(W_mu, np.float32), np.asarray(W_logstd, np.float32)], axis=1
    ).astype(BF16)
    wcat_t = np.ascontiguousarray(wcat.reshape(4, 128, DOUT))
    bias = np.concatenate(
        [np.asarray(b_mu, np.float32), np.asarray(b_logstd, np.float32)]
    ).astype(np.float32)
    bias_rep = np.tile(bias[None, :], (128, 1))
    iota_arr = np.tile(
        np.arange(128, dtype=np.float32).astype(BF16)[None, :], (128, 1)
    )

    nc = _build_program(prep)

    in_maps = []
    for c in range(NCORES):
        in_maps.append(
            {
                "xsT": xsT_cores[c],
                "wcat": wcat_t,
                "bias": bias_rep,
                "iota": iota_arr,
                "idx0": prep["idx_arrs"][0][c],
                "idx1": prep["idx_arrs"][1][c],
                "slot0": prep["slot_arrs"][0][c],
                "slot1": prep["slot_arrs"][1][c],
                "dinv_out": prep["dinv_out"][c],
                "nvalid": np.tile(prep["nvalid"][c][None, :], (128, 1)),
            }
        )

    trace = bool(os.environ.get("K_TRACE"))
    res = run_bass_kernel_spmd(
        nc, in_maps, core_ids=list(range(NCORES)), trace=trace
    )
    global LAST_RESULT
    LAST_RESULT = res
    if trace and res.exec_time_ns is not None:
        print(f"HW exec time: {res.exec_time_ns} ns")
    mu = np.concatenate([res.results[c]["out_mu"] for c in range(NCORES)], axis=0)
    ls = np.concatenate([res.results[c]["out_ls"] for c in range(NCORES)], axis=0)
    return (mu, ls)

